# revision 5
# baseline (speedup 1.0000x reference)
"""GCN graph-classification kernel for 8 Trainium2 NeuronCores.

Strategy (graph-aligned slot partitioning):
- Nodes are re-indexed into per-graph fixed-width slots (W slots/graph),
  so each core owns exactly G/8 graphs worth of contiguous slots. Every
  core then runs an IDENTICAL program (SPMD requirement) with per-core
  DATA (edge indices, norms) only.
- Per layer: x' = elu(D^-1/2 (A+I) D^-1/2 x W + b) computed aggregate-
  first: gather T[src] rows (T = dinv*x, folds the src factor), build a
  0/1 selection matrix S per 128-edge tile from dst offsets, accumulate
  S^T @ G into PSUM per 128-slot block (matmul = segment-sum), scale by
  dinv[dst], transpose, multiply by W, add bias, ELU.
- Tables for layers 1,2 are AllGathered across cores (each core computes
  its slot slice).
- Pooling: layer-2 epilogue transposes blocks into a [128, slots] strip
  initialized to -1e30; per-graph segment-max is then a fixed-width
  reduce_max per graph slot. Head (pooled @ Wl + bl, softmax) runs on
  device; cores each output their 125 graphs; host concatenates.
"""
import os
import numpy as np
import concourse.bass as bass
import concourse.mybir as mybir
from concourse.tile import TileContext
from concourse.bass_utils import run_bass_kernel_spmd
from concourse.masks import make_identity

N = 100000
E = 1600000
F = 128
H = 128
C = 10
G = 1000
NCORES = 8
GPC = G // NCORES          # graphs per core
P = 128

_MAXW_SPLIT = 1


def _split_waits(nc, maxw=_MAXW_SPLIT):
    """This walrus build accepts only 1 sync-wait on several instruction
    encodings; move excess waits onto preceding NoOps (same engine =>
    same queue => order preserved)."""
    cnt = 0
    for f in nc.m.functions:
        for bb in f.blocks:
            new_insts = []
            for inst in bb.instructions:
                si = inst.sync_info
                if si is not None and si.on_wait is not None and len(si.on_wait) > maxw:
                    waits = list(si.on_wait)
                    extra, keep = waits[:-maxw], waits[-maxw:]
                    for j in range(0, len(extra), maxw):
                        nop = mybir.InstNoOp(name=f"I-waitsplit-{cnt}")
                        cnt += 1
                        nop.engine = inst.engine
                        nop.sync_info = mybir.SyncInfo(on_wait=extra[j:j + maxw], on_update=[])
                        new_insts.append(nop)
                        nc.register_instruction(nop)
                    inst.sync_info = mybir.SyncInfo(on_wait=keep, on_update=list(si.on_update))
                new_insts.append(inst)
            bb.instructions[:] = new_insts
    return cnt


def _prep(x, edge_index, batch, W0, b0, W1, b1, W2, b2, Wl, bl):
    """Host-side sharding prep: slot layout, per-core edge tiles."""
    x = np.asarray(x, np.float32)
    ei = np.asarray(edge_index, np.int64)
    batch = np.asarray(batch, np.int64)

    # degrees incl self-loop (reference: segment_sum of ones over dst + loop)
    deg = np.bincount(ei[1], minlength=N).astype(np.float32) + 1.0
    dinv = 1.0 / np.sqrt(np.maximum(deg, 1.0))

    # graph sizes & slot width
    gcnt = np.bincount(batch, minlength=G).astype(np.int64)
    maxg = int(gcnt.max())
    W = ((maxg + 31) // 32) * 32 + 32          # fixed slot width per graph
    SPC_raw = GPC * W                          # slots per core before pad
    SPC = ((SPC_raw + P - 1) // P) * P         # pad to block multiple
    NB = SPC // P                              # blocks per core
    SLOTS = SPC * NCORES

    # node -> slot (batch sorted, nodes of graph g contiguous)
    gstart = np.zeros(G + 1, np.int64)
    np.cumsum(gcnt, out=gstart[1:])
    rank = np.arange(N, dtype=np.int64) - gstart[batch]
    core_of_g = batch // GPC
    slot = core_of_g * SPC + (batch - core_of_g * GPC) * W + rank

    # slot tables
    T0 = np.zeros((SLOTS, F), np.float32)
    T0[slot] = x * dinv[:, None]
    dinv_slot = np.ones(SLOTS, np.float32)
    dinv_slot[slot] = dinv
    pad_slot = np.full(SLOTS, -1e30, np.float32)
    pad_slot[slot] = 0.0

    # edges incl self-loops, in slot space
    src_s = np.concatenate([slot[ei[0]], slot])
    dst_s = np.concatenate([slot[ei[1]], slot])

    core_of_e = dst_s // SPC
    blk_of_e = (dst_s % SPC) // P
    dloc_of_e = dst_s % P

    # group edges by (core, block); per-block tile count = max over cores
    order = np.lexsort((blk_of_e, core_of_e))
    src_s, dst_s = src_s[order], dst_s[order]
    core_of_e, blk_of_e, dloc_of_e = core_of_e[order], blk_of_e[order], dloc_of_e[order]

    counts = np.zeros((NCORES, NB), np.int64)
    np.add.at(counts, (core_of_e, blk_of_e), 1)
    tpb = np.maximum((counts.max(axis=0) + P - 1) // P, 1)   # tiles per block (uniform across cores)
    TT = int(tpb.sum())                                      # total tiles per layer
    tilebase = np.zeros(NB + 1, np.int64)
    np.cumsum(tpb, out=tilebase[1:])

    idxs = np.zeros((NCORES, P, TT), np.int32)               # src slot per edge lane
    dls = np.full((NCORES, P, TT), -1.0, np.float32)         # dst offset in block (-1 => pad lane)
    eoff = np.zeros((NCORES, NB + 1), np.int64)
    np.cumsum(counts, axis=1, out=eoff[:, 1:])
    base_c = np.searchsorted(core_of_e, np.arange(NCORES))
    for c in range(NCORES):
        for b in range(NB):
            s0 = base_c[c] + eoff[c, b]
            cnt = counts[c, b]
            e_src = src_s[s0:s0 + cnt]
            e_dl = dloc_of_e[s0:s0 + cnt]
            ntile = int(tpb[b])
            buf_i = np.zeros(ntile * P, np.int32)
            buf_d = np.full(ntile * P, -1.0, np.float32)
            buf_i[:cnt] = e_src
            buf_d[:cnt] = e_dl
            cols = slice(int(tilebase[b]), int(tilebase[b + 1]))
            idxs[c, :, cols] = buf_i.reshape(ntile, P).T
            dls[c, :, cols] = buf_d.reshape(ntile, P).T

    dinvb = np.stack([
        dinv_slot[c * SPC:(c + 1) * SPC].reshape(NB, P).T for c in range(NCORES)
    ])                                                       # [NCORES, P, NB]
    padb = np.stack([
        pad_slot[c * SPC:(c + 1) * SPC].reshape(NB, P).T for c in range(NCORES)
    ])

    iota = np.broadcast_to(np.arange(P, dtype=np.float32), (P, P)).copy()
    b0r = np.broadcast_to(np.asarray(b0, np.float32), (P, H)).copy()
    b1r = np.broadcast_to(np.asarray(b1, np.float32), (P, H)).copy()
    b2r = np.broadcast_to(np.asarray(b2, np.float32), (P, H)).copy()
    blr = np.broadcast_to(np.asarray(bl, np.float32), (P, C)).copy()

    return dict(
        T0=T0, idxs=idxs, dls=dls, dinvb=dinvb, padb=padb, iota=iota,
        W0=np.asarray(W0, np.float32), W1=np.asarray(W1, np.float32),
        W2=np.asarray(W2, np.float32), Wl=np.asarray(Wl, np.float32),
        b0r=b0r, b1r=b1r, b2r=b2r, blr=blr,
        Wslot=W, SPC=SPC, NB=NB, SLOTS=SLOTS, TT=TT,
        tpb=tpb, tilebase=tilebase, gcnt=gcnt, bl=np.asarray(bl, np.float32),
    )


def _build(meta):
    f32 = mybir.dt.float32
    SPC, NB, SLOTS, TT, Wslot = meta["SPC"], meta["NB"], meta["SLOTS"], meta["TT"], meta["Wslot"]
    tpb, tilebase = meta["tpb"], meta["tilebase"]

    nc = bass.Bass(dynamic_dma_scratch_size=65536)
    t0 = nc.declare_dram_parameter("t0", [SLOTS, F], f32, isOutput=False)
    idxs_d = nc.declare_dram_parameter("idxs", [P, TT], mybir.dt.int32, isOutput=False)
    dls_d = nc.declare_dram_parameter("dls", [P, TT], f32, isOutput=False)
    dinv_d = nc.declare_dram_parameter("dinvb", [P, NB], f32, isOutput=False)
    pad_d = nc.declare_dram_parameter("padb", [P, NB], f32, isOutput=False)
    iota_d = nc.declare_dram_parameter("iota", [P, P], f32, isOutput=False)
    w_d = [nc.declare_dram_parameter(n, [H, H], f32, isOutput=False) for n in ("w0", "w1", "w2")]
    b_d = [nc.declare_dram_parameter(n, [P, H], f32, isOutput=False) for n in ("b0r", "b1r", "b2r")]
    wl_d = nc.declare_dram_parameter("wl", [H, C], f32, isOutput=False)
    blr_d = nc.declare_dram_parameter("blr", [P, C], f32, isOutput=False)
    out_d = nc.declare_dram_parameter("out", [GPC, C], f32, isOutput=True)


    rg = [list(range(NCORES))]
    AX = mybir.AxisListType.X
    OP = mybir.AluOpType

    with TileContext(nc) as tc:
        with tc.tile_pool(name="const", bufs=1) as cp, \
             tc.tile_pool(name="strip", bufs=1) as stp, \
             tc.tile_pool(name="gp", bufs=6) as gp, \
             tc.tile_pool(name="sp", bufs=4) as sp, \
             tc.tile_pool(name="ep", bufs=3) as ep, \
             tc.tile_pool(name="agg", bufs=2, space="PSUM") as aggp, \
             tc.tile_pool(name="tps", bufs=2, space="PSUM") as tpsp, \
             tc.tile_pool(name="mmp", bufs=2, space="PSUM") as mmp, \
             tc.tile_pool(name="dramp", bufs=1, space="DRAM") as dramp:

            tloc = [dramp.tile([SPC, F], f32, name=f"t{l}loc", tag=f"t{l}loc") for l in (1, 2)]
            tfull = [dramp.tile([SLOTS, F], f32, name=f"t{l}full", tag=f"t{l}full",
                                addr_space="Shared") for l in (1, 2)]

            ident = cp.tile([P, P], f32)
            make_identity(nc, ident[:])
            iota_sb = cp.tile([P, P], f32)
            nc.sync.dma_start(out=iota_sb[:], in_=iota_d[:])
            idxs_sb = cp.tile([P, TT], mybir.dt.int32)
            nc.sync.dma_start(out=idxs_sb[:], in_=idxs_d[:])
            dls_sb = cp.tile([P, TT], f32)
            nc.sync.dma_start(out=dls_sb[:], in_=dls_d[:])
            dinv_sb = cp.tile([P, NB], f32)
            nc.sync.dma_start(out=dinv_sb[:], in_=dinv_d[:])
            pad_sb = cp.tile([P, NB], f32)
            nc.sync.dma_start(out=pad_sb[:], in_=pad_d[:])
            w_sb, b_sb = [], []
            for l in range(3):
                wt = cp.tile([H, H], f32)
                nc.sync.dma_start(out=wt[:], in_=w_d[l][:])
                w_sb.append(wt)
                bt = cp.tile([P, H], f32)
                nc.sync.dma_start(out=bt[:], in_=b_d[l][:])
                b_sb.append(bt)
            wl_sb = cp.tile([H, C], f32)
            nc.sync.dma_start(out=wl_sb[:], in_=wl_d[:])
            blr_sb = cp.tile([P, C], f32)
            nc.sync.dma_start(out=blr_sb[:], in_=blr_d[:])

            strip = stp.tile([P, SPC], f32)
            nc.vector.memset(strip[:], -1e30)

            for layer in range(int(os.environ.get("K_LAYERS", "3"))):
                table = (t0, tfull[0], tfull[1])[layer]
                for b in range(NB):
                    k = int(tpb[b])
                    acc = aggp.tile([P, H], f32, tag="acc")
                    for t in range(k):
                        col = int(tilebase[b]) + t
                        g = gp.tile([P, F], f32, tag="g")
                        nc.gpsimd.indirect_dma_start(
                            out=g[:], out_offset=None, in_=table[:],
                            in_offset=bass.IndirectOffsetOnAxis(
                                ap=idxs_sb[:, col:col + 1], axis=0))
                        s = sp.tile([P, P], f32, tag="s")
                        nc.vector.tensor_tensor(
                            out=s[:], in0=iota_sb[:],
                            in1=dls_sb[:, col:col + 1].to_broadcast([P, P]),
                            op=OP.is_equal)
                        nc.tensor.matmul(out=acc[:], lhsT=s[:], rhs=g[:],
                                         start=(t == 0), stop=(t == k - 1))
                    # epilogue: scale by dinv[dst], transpose, @W, +b, ELU
                    aggs = ep.tile([P, H], f32, tag="aggs")
                    nc.vector.tensor_scalar(out=aggs[:], in0=acc[:],
                                            scalar1=dinv_sb[:, b:b + 1],
                                            scalar2=None, op0=OP.mult)
                    tp = tpsp.tile([P, H], f32, tag="tp")
                    nc.tensor.transpose(out=tp[:], in_=aggs[:], identity=ident[:])
                    aggt = ep.tile([P, H], f32, tag="aggt")
                    nc.vector.tensor_copy(out=aggt[:], in_=tp[:])
                    mm = mmp.tile([P, H], f32, tag="mm")
                    nc.tensor.matmul(out=mm[:], lhsT=aggt[:], rhs=w_sb[layer][:],
                                     start=True, stop=True)
                    xp = ep.tile([P, H], f32, tag="xp")
                    nc.vector.tensor_tensor(out=xp[:], in0=mm[:], in1=b_sb[layer][:], op=OP.add)
                    xn = ep.tile([P, H], f32, tag="xn")
                    nc.vector.tensor_scalar(out=xn[:], in0=xp[:], scalar1=0.0,
                                            scalar2=None, op0=OP.min)
                    en = ep.tile([P, H], f32, tag="en")
                    nc.scalar.activation(out=en[:], in_=xn[:],
                                         func=mybir.ActivationFunctionType.Exp)
                    xm = ep.tile([P, H], f32, tag="xm")
                    nc.vector.tensor_scalar(out=xm[:], in0=xp[:], scalar1=0.0,
                                            scalar2=-1.0, op0=OP.max, op1=OP.add)
                    xe = ep.tile([P, H], f32, tag="xe")
                    nc.vector.tensor_tensor(out=xe[:], in0=xm[:], in1=en[:], op=OP.add)
                    if layer < 2:
                        tn = ep.tile([P, H], f32, tag="tn")
                        nc.vector.tensor_scalar(out=tn[:], in0=xe[:],
                                                scalar1=dinv_sb[:, b:b + 1],
                                                scalar2=None, op0=OP.mult)
                        nc.sync.dma_start(out=tloc[layer][b * P:(b + 1) * P, :], in_=tn[:])
                    else:
                        xk = ep.tile([P, H], f32, tag="xk")
                        nc.vector.tensor_scalar(out=xk[:], in0=xe[:],
                                                scalar1=pad_sb[:, b:b + 1],
                                                scalar2=None, op0=OP.add)
                        tp2 = tpsp.tile([P, H], f32, tag="tp")
                        nc.tensor.transpose(out=tp2[:], in_=xk[:], identity=ident[:])
                        nc.vector.tensor_copy(out=strip[:, b * P:(b + 1) * P], in_=tp2[:])
                if layer < 2 and not os.environ.get("K_NOCC"):
                    nc.gpsimd.collective_compute(
                        "AllGather", OP.bypass, replica_groups=rg,
                        ins=[tloc[layer][:]], outs=[tfull[layer][:]])

            # pooling: fixed-width segment max per graph slot
            if os.environ.get("K_NOPOOL"):
                nc.sync.dma_start(out=out_d[:], in_=strip[:GPC, :C])
                raise SystemExit(0) if False else None
            pooled = cp.tile([P, GPC], f32)
            for s_i in range(GPC):
                nc.vector.reduce_max(out=pooled[:, s_i:s_i + 1],
                                     in_=strip[:, s_i * Wslot:(s_i + 1) * Wslot], axis=AX)
            # head: logits = pooled^T @ Wl + bl, softmax
            lg = mmp.tile([P, C], f32, tag="lg")
            nc.tensor.matmul(out=lg[:GPC, :], lhsT=pooled[:, :GPC], rhs=wl_sb[:],
                             start=True, stop=True)
            lo = cp.tile([P, C], f32)
            nc.vector.tensor_tensor(out=lo[:GPC], in0=lg[:GPC, :], in1=blr_sb[:GPC], op=OP.add)
            mx = cp.tile([P, 1], f32)
            nc.vector.reduce_max(out=mx[:GPC], in_=lo[:GPC], axis=AX)
            lo2 = cp.tile([P, C], f32)
            nc.vector.tensor_scalar(out=lo2[:GPC], in0=lo[:GPC], scalar1=mx[:GPC, :1],
                                    scalar2=None, op0=OP.subtract)
            ex = cp.tile([P, C], f32)
            nc.scalar.activation(out=ex[:GPC], in_=lo2[:GPC],
                                 func=mybir.ActivationFunctionType.Exp)
            sm = cp.tile([P, 1], f32)
            nc.vector.reduce_sum(out=sm[:GPC], in_=ex[:GPC], axis=AX)
            ri = cp.tile([P, 1], f32)
            nc.vector.reciprocal(out=ri[:GPC], in_=sm[:GPC])
            pr = cp.tile([P, C], f32)
            nc.vector.tensor_scalar(out=pr[:GPC], in0=ex[:GPC], scalar1=ri[:GPC, :1],
                                    scalar2=None, op0=OP.mult)
            nc.sync.dma_start(out=out_d[:], in_=pr[:GPC])

    _split_waits(nc)
    return nc


def kernel(x, edge_index, batch, W0, b0, W1, b1, W2, b2, Wl, bl):
    meta = _prep(x, edge_index, batch, W0, b0, W1, b1, W2, b2, Wl, bl)
    nc = _build(meta)
    in_maps = []
    for c in range(NCORES):
        in_maps.append({
            "t0": meta["T0"], "idxs": meta["idxs"][c], "dls": meta["dls"][c],
            "dinvb": meta["dinvb"][c], "padb": meta["padb"][c], "iota": meta["iota"],
            "w0": meta["W0"], "w1": meta["W1"], "w2": meta["W2"],
            "b0r": meta["b0r"], "b1r": meta["b1r"], "b2r": meta["b2r"],
            "wl": meta["Wl"], "blr": meta["blr"],
        })
    res = run_bass_kernel_spmd(nc, in_maps, core_ids=list(range(NCORES)))
    out = np.concatenate([res.results[c]["out"] for c in range(NCORES)], axis=0)
    # empty graphs (none in practice): reference yields softmax(bl)
    empty = meta["gcnt"] == 0
    if empty.any():
        e = np.exp(meta["bl"] - meta["bl"].max())
        out[empty] = e / e.sum()
    return out.astype(np.float32)


# revision 6
# speedup vs baseline: 3199.8423x; 3199.8423x over previous
"""GCN graph-classification kernel for 8 Trainium2 NeuronCores.

Strategy (graph-aligned slot partitioning):
- Nodes are re-indexed into per-graph fixed-width slots (W slots/graph),
  so each core owns exactly G/8 graphs worth of contiguous slots. Every
  core then runs an IDENTICAL program (SPMD requirement) with per-core
  DATA (edge indices, norms) only.
- Per layer: x' = elu(D^-1/2 (A+I) D^-1/2 x W + b) computed aggregate-
  first: gather T[src] rows (T = dinv*x, folds the src factor), build a
  0/1 selection matrix S per 128-edge tile from dst offsets, accumulate
  S^T @ G into PSUM per 128-slot block (matmul = segment-sum), scale by
  dinv[dst], transpose, multiply by W, add bias, ELU.
- Tables for layers 1,2 are AllGathered across cores (each core computes
  its slot slice).
- Pooling: layer-2 epilogue transposes blocks into a [128, slots] strip
  initialized to -1e30; per-graph segment-max is then a fixed-width
  reduce_max per graph slot. Head (pooled @ Wl + bl, softmax) runs on
  device; cores each output their 125 graphs; host concatenates.
"""
import os
import numpy as np
import concourse.bass as bass
import concourse.mybir as mybir
from concourse.tile import TileContext
from concourse.bass_utils import run_bass_kernel_spmd
from concourse.masks import make_identity

N = 100000
E = 1600000
F = 128
H = 128
C = 10
G = 1000
NCORES = 8
GPC = G // NCORES          # graphs per core
P = 128

_MAXW_SPLIT = 1


def _split_waits(nc, maxw=_MAXW_SPLIT):
    """This walrus build accepts only 1 sync-wait on several instruction
    encodings; move excess waits onto preceding NoOps (same engine =>
    same queue => order preserved)."""
    cnt = 0
    for f in nc.m.functions:
        for bb in f.blocks:
            new_insts = []
            for inst in bb.instructions:
                si = inst.sync_info
                if si is not None and si.on_wait is not None and len(si.on_wait) > maxw:
                    waits = list(si.on_wait)
                    extra, keep = waits[:-maxw], waits[-maxw:]
                    for j in range(0, len(extra), maxw):
                        nop = mybir.InstNoOp(name=f"I-waitsplit-{cnt}")
                        cnt += 1
                        nop.engine = inst.engine
                        nop.sync_info = mybir.SyncInfo(on_wait=extra[j:j + maxw], on_update=[])
                        new_insts.append(nop)
                        nc.register_instruction(nop)
                    inst.sync_info = mybir.SyncInfo(on_wait=keep, on_update=list(si.on_update))
                new_insts.append(inst)
            bb.instructions[:] = new_insts
    return cnt


def _prep(x, edge_index, batch, W0, b0, W1, b1, W2, b2, Wl, bl):
    """Host-side sharding prep: slot layout, per-core edge tiles."""
    x = np.asarray(x, np.float32)
    ei = np.asarray(edge_index, np.int64)
    batch = np.asarray(batch, np.int64)

    # degrees incl self-loop (reference: segment_sum of ones over dst + loop)
    deg = np.bincount(ei[1], minlength=N).astype(np.float32) + 1.0
    dinv = 1.0 / np.sqrt(np.maximum(deg, 1.0))

    # graph sizes & slot width
    gcnt = np.bincount(batch, minlength=G).astype(np.int64)
    maxg = int(gcnt.max())
    W = ((maxg + 31) // 32) * 32 + 32          # fixed slot width per graph
    SPC_raw = GPC * W                          # slots per core before pad
    SPC = ((SPC_raw + P - 1) // P) * P         # pad to block multiple
    NB = SPC // P                              # blocks per core
    SLOTS = SPC * NCORES

    # node -> slot (batch sorted, nodes of graph g contiguous)
    gstart = np.zeros(G + 1, np.int64)
    np.cumsum(gcnt, out=gstart[1:])
    rank = np.arange(N, dtype=np.int64) - gstart[batch]
    core_of_g = batch // GPC
    slot = core_of_g * SPC + (batch - core_of_g * GPC) * W + rank

    # slot tables
    T0 = np.zeros((SLOTS, F), np.float32)
    T0[slot] = x * dinv[:, None]
    dinv_slot = np.ones(SLOTS, np.float32)
    dinv_slot[slot] = dinv
    pad_slot = np.full(SLOTS, -1e30, np.float32)
    pad_slot[slot] = 0.0

    # edges incl self-loops, in slot space
    src_s = np.concatenate([slot[ei[0]], slot])
    dst_s = np.concatenate([slot[ei[1]], slot])

    core_of_e = dst_s // SPC
    blk_of_e = (dst_s % SPC) // P
    dloc_of_e = dst_s % P

    # group edges by (core, block); per-block tile count = max over cores
    order = np.lexsort((blk_of_e, core_of_e))
    src_s, dst_s = src_s[order], dst_s[order]
    core_of_e, blk_of_e, dloc_of_e = core_of_e[order], blk_of_e[order], dloc_of_e[order]

    counts = np.zeros((NCORES, NB), np.int64)
    np.add.at(counts, (core_of_e, blk_of_e), 1)
    tpb = np.maximum((counts.max(axis=0) + P - 1) // P, 1)   # tiles per block (uniform across cores)
    TT = int(tpb.sum())                                      # total tiles per layer
    tilebase = np.zeros(NB + 1, np.int64)
    np.cumsum(tpb, out=tilebase[1:])

    idxs = np.zeros((NCORES, P, TT), np.int32)               # src slot per edge lane
    dls = np.full((NCORES, P, TT), -1.0, np.float32)         # dst offset in block (-1 => pad lane)
    eoff = np.zeros((NCORES, NB + 1), np.int64)
    np.cumsum(counts, axis=1, out=eoff[:, 1:])
    base_c = np.searchsorted(core_of_e, np.arange(NCORES))
    for c in range(NCORES):
        for b in range(NB):
            s0 = base_c[c] + eoff[c, b]
            cnt = counts[c, b]
            e_src = src_s[s0:s0 + cnt]
            e_dl = dloc_of_e[s0:s0 + cnt]
            ntile = int(tpb[b])
            buf_i = np.zeros(ntile * P, np.int32)
            buf_d = np.full(ntile * P, -1.0, np.float32)
            buf_i[:cnt] = e_src
            buf_d[:cnt] = e_dl
            cols = slice(int(tilebase[b]), int(tilebase[b + 1]))
            idxs[c, :, cols] = buf_i.reshape(ntile, P).T
            dls[c, :, cols] = buf_d.reshape(ntile, P).T

    dinvb = np.stack([
        dinv_slot[c * SPC:(c + 1) * SPC].reshape(NB, P).T for c in range(NCORES)
    ])                                                       # [NCORES, P, NB]
    padb = np.stack([
        pad_slot[c * SPC:(c + 1) * SPC].reshape(NB, P).T for c in range(NCORES)
    ])

    iota = np.broadcast_to(np.arange(P, dtype=np.float32), (P, P)).copy()
    b0r = np.broadcast_to(np.asarray(b0, np.float32), (P, H)).copy()
    b1r = np.broadcast_to(np.asarray(b1, np.float32), (P, H)).copy()
    b2r = np.broadcast_to(np.asarray(b2, np.float32), (P, H)).copy()
    blr = np.broadcast_to(np.asarray(bl, np.float32), (P, C)).copy()

    return dict(
        T0=T0, idxs=idxs, dls=dls, dinvb=dinvb, padb=padb, iota=iota,
        W0=np.asarray(W0, np.float32), W1=np.asarray(W1, np.float32),
        W2=np.asarray(W2, np.float32), Wl=np.asarray(Wl, np.float32),
        b0r=b0r, b1r=b1r, b2r=b2r, blr=blr,
        Wslot=W, SPC=SPC, NB=NB, SLOTS=SLOTS, TT=TT,
        tpb=tpb, tilebase=tilebase, gcnt=gcnt, bl=np.asarray(bl, np.float32),
    )


def _build(meta):
    f32 = mybir.dt.float32
    SPC, NB, SLOTS, TT, Wslot = meta["SPC"], meta["NB"], meta["SLOTS"], meta["TT"], meta["Wslot"]
    tpb, tilebase = meta["tpb"], meta["tilebase"]

    nc = bass.Bass(dynamic_dma_scratch_size=65536)
    t0 = nc.declare_dram_parameter("t0", [SLOTS, F], f32, isOutput=False)
    idxs_d = nc.declare_dram_parameter("idxs", [P, TT], mybir.dt.int32, isOutput=False)
    dls_d = nc.declare_dram_parameter("dls", [P, TT], f32, isOutput=False)
    dinv_d = nc.declare_dram_parameter("dinvb", [P, NB], f32, isOutput=False)
    pad_d = nc.declare_dram_parameter("padb", [P, NB], f32, isOutput=False)
    iota_d = nc.declare_dram_parameter("iota", [P, P], f32, isOutput=False)
    w_d = [nc.declare_dram_parameter(n, [H, H], f32, isOutput=False) for n in ("w0", "w1", "w2")]
    b_d = [nc.declare_dram_parameter(n, [P, H], f32, isOutput=False) for n in ("b0r", "b1r", "b2r")]
    wl_d = nc.declare_dram_parameter("wl", [H, C], f32, isOutput=False)
    blr_d = nc.declare_dram_parameter("blr", [P, C], f32, isOutput=False)
    out_d = nc.declare_dram_parameter("out", [GPC, C], f32, isOutput=True)


    rg = [list(range(NCORES))]
    AX = mybir.AxisListType.X
    OP = mybir.AluOpType

    with TileContext(nc) as tc:
        with tc.tile_pool(name="const", bufs=1) as cp, \
             tc.tile_pool(name="strip", bufs=1) as stp, \
             tc.tile_pool(name="gp", bufs=6) as gp, \
             tc.tile_pool(name="sp", bufs=4) as sp, \
             tc.tile_pool(name="ep", bufs=3) as ep, \
             tc.tile_pool(name="agg", bufs=2, space="PSUM") as aggp, \
             tc.tile_pool(name="tps", bufs=2, space="PSUM") as tpsp, \
             tc.tile_pool(name="mmp", bufs=2, space="PSUM") as mmp, \
             tc.tile_pool(name="dramp", bufs=1, space="DRAM") as dramp:

            tloc = [dramp.tile([SPC, F], f32, name=f"t{l}loc", tag=f"t{l}loc") for l in (1, 2)]
            tfull = [dramp.tile([SLOTS, F], f32, name=f"t{l}full", tag=f"t{l}full",
                                addr_space="Shared") for l in (1, 2)]

            ident = cp.tile([P, P], f32)
            make_identity(nc, ident[:])
            iota_sb = cp.tile([P, P], f32)
            nc.sync.dma_start(out=iota_sb[:], in_=iota_d[:])
            idxs_sb = cp.tile([P, TT], mybir.dt.int32)
            nc.sync.dma_start(out=idxs_sb[:], in_=idxs_d[:])
            dls_sb = cp.tile([P, TT], f32)
            nc.sync.dma_start(out=dls_sb[:], in_=dls_d[:])
            dinv_sb = cp.tile([P, NB], f32)
            nc.sync.dma_start(out=dinv_sb[:], in_=dinv_d[:])
            pad_sb = cp.tile([P, NB], f32)
            nc.sync.dma_start(out=pad_sb[:], in_=pad_d[:])
            w_sb, b_sb = [], []
            for l in range(3):
                wt = cp.tile([H, H], f32)
                nc.sync.dma_start(out=wt[:], in_=w_d[l][:])
                w_sb.append(wt)
                bt = cp.tile([P, H], f32)
                nc.sync.dma_start(out=bt[:], in_=b_d[l][:])
                b_sb.append(bt)
            wl_sb = cp.tile([H, C], f32)
            nc.sync.dma_start(out=wl_sb[:], in_=wl_d[:])
            blr_sb = cp.tile([P, C], f32)
            nc.sync.dma_start(out=blr_sb[:], in_=blr_d[:])

            strip = stp.tile([P, SPC], f32)
            nc.vector.memset(strip[:], -1e30)

            for layer in range(int(os.environ.get("K_LAYERS", "3"))):
                table = (t0, tfull[0], tfull[1])[layer]
                for b in range(NB):
                    k = int(tpb[b])
                    acc = aggp.tile([P, H], f32, tag="acc")
                    for t in range(k):
                        col = int(tilebase[b]) + t
                        g = gp.tile([P, F], f32, tag="g")
                        nc.gpsimd.indirect_dma_start(
                            out=g[:], out_offset=None, in_=table[:],
                            in_offset=bass.IndirectOffsetOnAxis(
                                ap=idxs_sb[:, col:col + 1], axis=0))
                        s = sp.tile([P, P], f32, tag="s")
                        nc.vector.tensor_tensor(
                            out=s[:], in0=iota_sb[:],
                            in1=dls_sb[:, col:col + 1].to_broadcast([P, P]),
                            op=OP.is_equal)
                        nc.tensor.matmul(out=acc[:], lhsT=s[:], rhs=g[:],
                                         start=(t == 0), stop=(t == k - 1))
                    # epilogue: scale by dinv[dst], transpose, @W, +b, ELU
                    aggs = ep.tile([P, H], f32, tag="aggs")
                    nc.vector.tensor_scalar(out=aggs[:], in0=acc[:],
                                            scalar1=dinv_sb[:, b:b + 1],
                                            scalar2=None, op0=OP.mult)
                    tp = tpsp.tile([P, H], f32, tag="tp")
                    nc.tensor.transpose(out=tp[:], in_=aggs[:], identity=ident[:])
                    aggt = ep.tile([P, H], f32, tag="aggt")
                    nc.vector.tensor_copy(out=aggt[:], in_=tp[:])
                    mm = mmp.tile([P, H], f32, tag="mm")
                    nc.tensor.matmul(out=mm[:], lhsT=aggt[:], rhs=w_sb[layer][:],
                                     start=True, stop=True)
                    xp = ep.tile([P, H], f32, tag="xp")
                    nc.vector.tensor_tensor(out=xp[:], in0=mm[:], in1=b_sb[layer][:], op=OP.add)
                    xn = ep.tile([P, H], f32, tag="xn")
                    nc.vector.tensor_scalar(out=xn[:], in0=xp[:], scalar1=0.0,
                                            scalar2=None, op0=OP.min)
                    en = ep.tile([P, H], f32, tag="en")
                    nc.scalar.activation(out=en[:], in_=xn[:],
                                         func=mybir.ActivationFunctionType.Exp)
                    xm = ep.tile([P, H], f32, tag="xm")
                    nc.vector.tensor_scalar(out=xm[:], in0=xp[:], scalar1=0.0,
                                            scalar2=-1.0, op0=OP.max, op1=OP.add)
                    xe = ep.tile([P, H], f32, tag="xe")
                    nc.vector.tensor_tensor(out=xe[:], in0=xm[:], in1=en[:], op=OP.add)
                    if layer < 2:
                        tn = ep.tile([P, H], f32, tag="tn")
                        nc.vector.tensor_scalar(out=tn[:], in0=xe[:],
                                                scalar1=dinv_sb[:, b:b + 1],
                                                scalar2=None, op0=OP.mult)
                        nc.sync.dma_start(out=tloc[layer][b * P:(b + 1) * P, :], in_=tn[:])
                    else:
                        xk = ep.tile([P, H], f32, tag="xk")
                        nc.vector.tensor_scalar(out=xk[:], in0=xe[:],
                                                scalar1=pad_sb[:, b:b + 1],
                                                scalar2=None, op0=OP.add)
                        tp2 = tpsp.tile([P, H], f32, tag="tp")
                        nc.tensor.transpose(out=tp2[:], in_=xk[:], identity=ident[:])
                        nc.vector.tensor_copy(out=strip[:, b * P:(b + 1) * P], in_=tp2[:])
                if layer < 2 and not os.environ.get("K_NOCC"):
                    nc.gpsimd.collective_compute(
                        "AllGather", OP.bypass, replica_groups=rg,
                        ins=[tloc[layer][:]], outs=[tfull[layer][:]])

            # pooling: fixed-width segment max per graph slot
            if os.environ.get("K_NOPOOL"):
                nc.sync.dma_start(out=out_d[:], in_=strip[:GPC, :C])
                raise SystemExit(0) if False else None
            pooled = cp.tile([P, GPC], f32)
            for s_i in range(GPC):
                nc.vector.reduce_max(out=pooled[:, s_i:s_i + 1],
                                     in_=strip[:, s_i * Wslot:(s_i + 1) * Wslot], axis=AX)
            # head: logits = pooled^T @ Wl + bl, softmax
            lg = mmp.tile([P, C], f32, tag="lg")
            nc.tensor.matmul(out=lg[:GPC, :], lhsT=pooled[:, :GPC], rhs=wl_sb[:],
                             start=True, stop=True)
            lo = cp.tile([P, C], f32)
            nc.vector.tensor_tensor(out=lo[:GPC], in0=lg[:GPC, :], in1=blr_sb[:GPC], op=OP.add)
            mx = cp.tile([P, 1], f32)
            nc.vector.reduce_max(out=mx[:GPC], in_=lo[:GPC], axis=AX)
            lo2 = cp.tile([P, C], f32)
            nc.vector.tensor_scalar(out=lo2[:GPC], in0=lo[:GPC], scalar1=mx[:GPC, :1],
                                    scalar2=None, op0=OP.subtract)
            ex = cp.tile([P, C], f32)
            nc.scalar.activation(out=ex[:GPC], in_=lo2[:GPC],
                                 func=mybir.ActivationFunctionType.Exp)
            sm = cp.tile([P, 1], f32)
            nc.vector.reduce_sum(out=sm[:GPC], in_=ex[:GPC], axis=AX)
            ri = cp.tile([P, 1], f32)
            nc.vector.reciprocal(out=ri[:GPC], in_=sm[:GPC])
            pr = cp.tile([P, C], f32)
            nc.vector.tensor_scalar(out=pr[:GPC], in0=ex[:GPC], scalar1=ri[:GPC, :1],
                                    scalar2=None, op0=OP.mult)
            nc.sync.dma_start(out=out_d[:], in_=pr[:GPC])

    _split_waits(nc)
    return nc


_BUILD_CACHE = {}


def kernel(x, edge_index, batch, W0, b0, W1, b1, W2, b2, Wl, bl):
    meta = _prep(x, edge_index, batch, W0, b0, W1, b1, W2, b2, Wl, bl)
    # program structure depends only on (SPC, TT, tpb); cache across calls
    key = (meta["SPC"], meta["TT"], meta["tpb"].tobytes())
    nc = _BUILD_CACHE.get(key)
    if nc is None:
        nc = _build(meta)
        _BUILD_CACHE[key] = nc
    in_maps = []
    for c in range(NCORES):
        in_maps.append({
            "t0": meta["T0"], "idxs": meta["idxs"][c], "dls": meta["dls"][c],
            "dinvb": meta["dinvb"][c], "padb": meta["padb"][c], "iota": meta["iota"],
            "w0": meta["W0"], "w1": meta["W1"], "w2": meta["W2"],
            "b0r": meta["b0r"], "b1r": meta["b1r"], "b2r": meta["b2r"],
            "wl": meta["Wl"], "blr": meta["blr"],
        })
    res = run_bass_kernel_spmd(nc, in_maps, core_ids=list(range(NCORES)))
    out = np.concatenate([res.results[c]["out"] for c in range(NCORES)], axis=0)
    # empty graphs (none in practice): reference yields softmax(bl)
    empty = meta["gcnt"] == 0
    if empty.any():
        e = np.exp(meta["bl"] - meta["bl"].max())
        out[empty] = e / e.sum()
    return out.astype(np.float32)
